# revision 1
# baseline (speedup 1.0000x reference)
"""Trainium2 Bass kernel for an 8-expert top-2 SwiGLU MoE (expert parallelism).

Strategy (8 NeuronCores, one expert per core):
  - Every core receives the full token set, the gate, and ITS expert's weights
    (pre-transposed to feature-major on the host as part of input marshaling).
  - On device, each core:
      1. computes gating logits for all 8192 tokens in exact fp32 on the PE,
      2. finds the top-2 experts per token (vector.max), derives the softmax
         renormalized weight for its own expert, and a routed-token mask,
      3. turns the mask into compact slot indices via matmul prefix-sums,
      4. scatters (token_id, weight) pairs into a compact table with
         OOB-skipping indirect DMA, gathers the routed token rows,
      5. runs the SwiGLU FFN (x@w1ᵀ, x@w3ᵀ, silu·mul, @w2ᵀ) in float32r
         (full-rate PE) over the compacted tokens,
      6. scales by the routing weight and writes token-major output rows.
  - The host adds each core's rows into the full output (expert-parallel
    combine; each token appears in at most K=2 cores' lists).

Self-contained: hardcodes shapes for x[4,2048,1024], 8 experts, H=2816, top-2.
"""
import sys

sys.path.insert(0, "/opt/trn_rl_repo")

import numpy as np

# ---------------------------------------------------------------- config
B, S, D = 4, 2048, 1024
T = B * S                # 8192 tokens
E = 8                    # experts == cores
H = 2816
K = 2
P = 128
NB = T // P              # 64 token blocks (token = 128*b + p)
C = 2560                 # per-expert slot capacity (mean 2048, ~12 sigma margin)
NG = C // P              # 20 slot tiles
HT = H // P              # 22
DT = D // P              # 8
GATE_CHUNK = 1024
# FFN super-chunks of the slot range: (start, len); slices of 512 inside.
CHUNKS = [(0, 1024), (1024, 1024), (2048, 512)]
SLICE = 512

_cache = {}


def _build():
    import concourse.bass as bass
    import concourse.bacc as bacc
    import concourse.mybir as mybir
    import concourse.tile as tile

    f32 = mybir.dt.float32
    f32r = mybir.dt.float32r
    i32 = mybir.dt.int32
    Alu = mybir.AluOpType
    Act = mybir.ActivationFunctionType

    nc = bacc.Bacc("TRN2", target_bir_lowering=False, debug=False)

    x_d = nc.dram_tensor("x", [T, D], f32, kind="ExternalInput")
    xT_d = nc.dram_tensor("xT", [D, T], f32, kind="ExternalInput")
    gwT_d = nc.dram_tensor("gwT", [D, E], f32, kind="ExternalInput")
    w1T_d = nc.dram_tensor("w1T", [D, H], f32r, kind="ExternalInput")
    w3T_d = nc.dram_tensor("w3T", [D, H], f32r, kind="ExternalInput")
    w2T_d = nc.dram_tensor("w2T", [H, D], f32r, kind="ExternalInput")
    esel_d = nc.dram_tensor("esel", [P, E], f32, kind="ExternalInput")
    uexc_d = nc.dram_tensor("uexc", [P, P], f32, kind="ExternalInput")
    onesc_d = nc.dram_tensor("ones_col", [P, 1], f32, kind="ExternalInput")
    onesr_d = nc.dram_tensor("ones_row", [1, P], f32, kind="ExternalInput")
    iota_d = nc.dram_tensor("iota", [P, NB], i32, kind="ExternalInput")
    ident_d = nc.dram_tensor("ident", [P, P], f32, kind="ExternalInput")

    idw_d = nc.dram_tensor("idw", [C, 2], i32, kind="ExternalOutput")
    cnt_d = nc.dram_tensor("cnt", [1, 1], f32, kind="ExternalOutput")
    y_d = nc.dram_tensor("y_rows", [C, D], f32, kind="ExternalOutput")

    with tile.TileContext(nc) as tc:
        with tc.tile_pool(name="persist", bufs=1) as sp:
            # --- constants ---
            esel = sp.tile([P, E], f32)
            nc.sync.dma_start(out=esel[:], in_=esel_d[:])
            uexc = sp.tile([P, P], f32)
            nc.sync.dma_start(out=uexc[:], in_=uexc_d[:])
            onesc = sp.tile([P, 1], f32)
            nc.sync.dma_start(out=onesc[:], in_=onesc_d[:])
            onesr = sp.tile([1, P], f32)
            nc.sync.dma_start(out=onesr[:], in_=onesr_d[:])
            iota = sp.tile([P, NB], i32)
            nc.sync.dma_start(out=iota[:], in_=iota_d[:])
            ident = sp.tile([P, P], f32)
            nc.sync.dma_start(out=ident[:], in_=ident_d[:])
            gw = sp.tile([P, DT, E], f32)
            nc.sync.dma_start(out=gw[:], in_=gwT_d[:].rearrange("(k p) e -> p k e", p=P))

            # PE wait-absorber: matmul codegen allows a single sync wait, so
            # before any matmul that would need 2+ waits we make the PE observe
            # the extra semaphores through a tiny dummy matmul.
            dummy_ps = None

            def pe_touch(ap):
                # ap: [1, 1..2] SBUF region; result is garbage, absorbs one sem wait
                n = ap.shape[-1]
                nc.tensor.matmul(dummy_ps[0:1, 0:n], lhsT=ap[:, 0:1], rhs=ap,
                                 start=True, stop=True, skip_group_check=True)

            scores = sp.tile([P, NB * E], f32)     # [p, b*E+e] logits
            mx_all = sp.tile([P, NB * 8], f32)     # per-block top-8 (descending)
            se = sp.tile([P, NB], f32)

            # ---------------- stage 1: gating logits (exact fp32) ----------------
            with tc.tile_pool(name="gpsum", bufs=2, space="PSUM") as ppg, \
                 tc.tile_pool(name="gsb", bufs=3) as sg:
                dummy_ps = ppg.tile([1, 2], f32, tag="dummy", bufs=1)
                pe_touch(gw[0:1, 0, 0:2])
                pe_touch(ident[0:1, 0:2])
                pe_touch(uexc[0:1, 0:2])
                pe_touch(onesc[0:1, 0:1])
                pe_touch(onesr[0:1, 0:2])
                NJ = T // GATE_CHUNK
                BPC = GATE_CHUNK // P          # blocks per gating chunk
                incl_all = sp.tile([1, NB], f32)
                for j in range(NJ):
                    xt = sg.tile([P, DT, GATE_CHUNK], f32, tag="xt", bufs=2)
                    nc.sync.dma_start(
                        out=xt[:],
                        in_=xT_d[:].rearrange("(k p) t -> p k t", p=P)[:, :, j * GATE_CHUNK:(j + 1) * GATE_CHUNK])
                    ps = ppg.tile([E, GATE_CHUNK], f32, tag="ps", space="PSUM")
                    for h0 in range(0, GATE_CHUNK, 512):
                        for k in range(DT):
                            nc.tensor.matmul(ps[:, h0:h0 + 512], lhsT=gw[:, k, :],
                                             rhs=xt[:, k, h0:h0 + 512],
                                             start=(k == 0), stop=(k == DT - 1))
                    sc_sb = sg.tile([E, GATE_CHUNK], f32, tag="sc")
                    nc.vector.tensor_copy(out=sc_sb[:], in_=ps[:])
                    for i in range(BPC):
                        b = j * BPC + i
                        pst = ppg.tile([P, E], f32, tag="pst", space="PSUM")
                        nc.tensor.transpose(out=pst[:], in_=sc_sb[:, i * P:(i + 1) * P],
                                            identity=ident[0:E, 0:E])
                        nc.vector.tensor_copy(out=scores[:, b * E:(b + 1) * E], in_=pst[:])
                        blk = scores[:, b * E:(b + 1) * E]
                        nc.vector.max(out=mx_all[:, b * 8:(b + 1) * 8], in_=blk)
                        t8 = sg.tile([P, E], f32, tag="t8")
                        nc.vector.tensor_tensor(out=t8[:], in0=blk, in1=esel[:], op=Alu.mult)
                        nc.vector.reduce_sum(out=se[:, b:b + 1], in_=t8[:], axis=mybir.AxisListType.X)

                    # ---- routing for this chunk's BPC blocks (overlaps next chunk's PE) ----
                    b0 = j * BPC
                    mx3 = mx_all[:].rearrange("p (b e) -> p b e", e=8)
                    m1j = mx3[:, b0:b0 + BPC, 0]
                    m2j = mx3[:, b0:b0 + BPC, 1]
                    sej = se[:, b0:b0 + BPC]
                    dlt = sg.tile([P, BPC], f32, tag="dlt")
                    nc.vector.tensor_sub(out=dlt[:], in0=m2j, in1=m1j)
                    ed = sg.tile([P, BPC], f32, tag="ed")
                    nc.scalar.activation(out=ed[:], in_=dlt[:], func=Act.Exp)
                    den = sg.tile([P, BPC], f32, tag="den")
                    nc.vector.tensor_scalar_add(den[:], ed[:], 1.0)
                    wtop = sg.tile([P, BPC], f32, tag="wtop")
                    nc.vector.reciprocal(out=wtop[:], in_=den[:])
                    wsec = sg.tile([P, BPC], f32, tag="wsec")
                    nc.vector.tensor_scalar(out=wsec[:], in0=wtop[:], scalar1=-1.0, scalar2=1.0,
                                            op0=Alu.mult, op1=Alu.add)
                    istop = sg.tile([P, BPC], f32, tag="istop")
                    nc.vector.tensor_tensor(out=istop[:], in0=sej, in1=m1j, op=Alu.is_ge)
                    wdiff = sg.tile([P, BPC], f32, tag="wdiff")
                    nc.vector.tensor_sub(out=wdiff[:], in0=wtop[:], in1=wsec[:])
                    wE = sg.tile([P, BPC], f32, tag="wE")
                    nc.vector.tensor_tensor(out=wE[:], in0=istop[:], in1=wdiff[:], op=Alu.mult)
                    nc.vector.tensor_add(out=wE[:], in0=wE[:], in1=wsec[:])
                    maskj = sg.tile([P, BPC], f32, tag="maskj")
                    nc.vector.tensor_tensor(out=maskj[:], in0=sej, in1=m2j, op=Alu.is_ge)

                    pslot = ppg.tile([P, BPC], f32, tag="pslot", space="PSUM", bufs=1)
                    nc.tensor.matmul(pslot[:], lhsT=uexc[:], rhs=maskj[:], start=True, stop=False)
                    ptot = ppg.tile([1, BPC], f32, tag="dummy", space="PSUM", bufs=1)
                    nc.tensor.matmul(ptot[:], lhsT=onesc[:], rhs=maskj[:], start=True, stop=True)
                    tot = sg.tile([1, BPC], f32, tag="tot")
                    nc.vector.tensor_copy(out=tot[:], in_=ptot[:])
                    init = 0.0 if j == 0 else incl_all[:, b0 - 1:b0]
                    nc.vector.tensor_tensor_scan(incl_all[:, b0:b0 + BPC], tot[:], tot[:], init,
                                                 op0=Alu.add, op1=Alu.bypass)
                    excl = sg.tile([1, BPC], f32, tag="excl")
                    nc.vector.tensor_sub(out=excl[:], in0=incl_all[:, b0:b0 + BPC], in1=tot[:])
                    nc.tensor.matmul(pslot[:], lhsT=onesr[:], rhs=excl[:], start=False, stop=True)
                    slot_f = sg.tile([P, BPC], f32, tag="slot_f")
                    nc.vector.tensor_copy(out=slot_f[:], in_=pslot[:])
                    off_f = sg.tile([P, BPC], f32, tag="off_f")
                    nc.vector.tensor_scalar(out=off_f[:], in0=maskj[:], scalar1=-1e6, scalar2=1e6,
                                            op0=Alu.mult, op1=Alu.add)
                    slot_oob = sg.tile([P, BPC], f32, tag="slot_oob")
                    nc.vector.tensor_add(out=slot_oob[:], in0=slot_f[:], in1=off_f[:])
                    slot_i = sg.tile([P, BPC], i32, tag="slot_i")
                    nc.vector.tensor_copy(out=slot_i[:], in_=slot_oob[:])
                    iw = sg.tile([P, 2 * BPC], i32, tag="iw")
                    iw3 = iw[:].rearrange("p (b two) -> p b two", two=2)
                    nc.vector.tensor_copy(out=iw3[:, :, 0], in_=iota[:, b0:b0 + BPC])
                    nc.vector.tensor_copy(out=iw3[:, :, 1], in_=wE[:].bitcast(i32))
                    for i in range(BPC):
                        nc.gpsimd.indirect_dma_start(
                            out=idw_d[:], out_offset=bass.IndirectOffsetOnAxis(ap=slot_i[:, i:i + 1], axis=0),
                            in_=iw[:, 2 * i:2 * i + 2], in_offset=None,
                            bounds_check=C - 1, oob_is_err=False)

                cnt_sb = sg.tile([1, 1], f32, tag="cnt")
                nc.vector.tensor_copy(out=cnt_sb[:], in_=incl_all[:, NB - 1:NB])
                nc.sync.dma_start(out=cnt_d[:], in_=cnt_sb[:])

            # ---------------- stage 3+4: per super-chunk gather + FFN ----------------
            h_all = [sp.tile([P, 1024], f32r, tag=f"h{ht}", name=f"h{ht}") for ht in range(HT)]
            xgT = [sp.tile([P, 1024], f32r, tag=f"xgT{k}", name=f"xgT{k}") for k in range(DT)]
            idw_sb = [sp.tile([P, 2], i32, tag=f"idw{g}", name=f"idw{g}") for g in range(8)]

            for (c0, clen) in CHUNKS:
                ngc = clen // P
                nsl = clen // SLICE if clen % SLICE == 0 else clen // SLICE + 1
                slices = [(s * SLICE, min(SLICE, clen - s * SLICE)) for s in range(nsl)]

                # gather + transpose to feature-major
                with tc.tile_pool(name="gat_ps", bufs=2, space="PSUM") as ppt, \
                     tc.tile_pool(name="gat_sb", bufs=3) as sgt:
                    dummy_ps = ppt.tile([1, 2], f32, tag="dummy", bufs=1)
                    for g in range(ngc):
                        gabs = c0 // P + g
                        nc.sync.dma_start(out=idw_sb[g][:], in_=idw_d[P * gabs:P * (gabs + 1), :])
                        xg = sgt.tile([P, D], f32, tag="xg")
                        nc.gpsimd.indirect_dma_start(
                            out=xg[:], out_offset=None, in_=x_d[:],
                            in_offset=bass.IndirectOffsetOnAxis(ap=idw_sb[g][:, 0:1], axis=0))
                        for k in range(DT):
                            pst = ppt.tile([P, P], f32, tag="pst", space="PSUM", bufs=4)
                            nc.tensor.transpose(out=pst[:], in_=xg[:, P * k:P * (k + 1)],
                                                identity=ident[:])
                            nc.vector.tensor_copy(out=xgT[k][:, g * P:(g + 1) * P], in_=pst[:])

                # FFN pass 1: h = silu(x@w1T) * (x@w3T)
                with tc.tile_pool(name="p1_ps", bufs=2, space="PSUM") as pp1, \
                     tc.tile_pool(name="p1_sb", bufs=3) as s1:
                    dummy_ps = pp1.tile([1, 2], f32, tag="dummy", bufs=1)
                    g_s0 = min(ngc, SLICE // P) - 1   # last slot-tile of slice 0
                    for k in range(DT):
                        pe_touch(xgT[k][0:1, g_s0 * P:g_s0 * P + 2])
                    prev_silu = None
                    for ht in range(HT):
                        w1b = s1.tile([P, DT, P], f32r, tag="w1b")
                        nc.sync.dma_start(
                            out=w1b[:],
                            in_=w1T_d[:].rearrange("(k p) h -> p k h", p=P)[:, :, ht * P:(ht + 1) * P])
                        w3b = s1.tile([P, DT, P], f32r, tag="w3b")
                        nc.sync.dma_start(
                            out=w3b[:],
                            in_=w3T_d[:].rearrange("(k p) h -> p k h", p=P)[:, :, ht * P:(ht + 1) * P])
                        for (s0, sl) in slices:
                            ph1 = pp1.tile([P, SLICE], f32, tag="ph1", space="PSUM")
                            ph3 = pp1.tile([P, SLICE], f32, tag="ph3", space="PSUM")
                            for k in range(DT):
                                nc.tensor.matmul(ph1[:, :sl], lhsT=w1b[:, k, :],
                                                 rhs=xgT[k][:, s0:s0 + sl],
                                                 start=(k == 0), stop=(k == DT - 1))
                            for k in range(DT):
                                nc.tensor.matmul(ph3[:, :sl], lhsT=w3b[:, k, :],
                                                 rhs=xgT[k][:, s0:s0 + sl],
                                                 start=(k == 0), stop=(k == DT - 1))
                            silu = s1.tile([P, SLICE], f32, tag="silu")
                            nc.scalar.activation(out=silu[:, :sl], in_=ph1[:, :sl], func=Act.Silu)
                            nc.vector.tensor_tensor(out=h_all[ht][:, s0:s0 + sl],
                                                    in0=silu[:, :sl], in1=ph3[:, :sl], op=Alu.mult)
                            if prev_silu is not None:
                                pe_touch(prev_silu)
                            prev_silu = silu[0:1, 0:2]

                # FFN pass 2: y = h @ w2T, transpose back, scale by routing weight
                with tc.tile_pool(name="p2_ps", bufs=2, space="PSUM") as pp2, \
                     tc.tile_pool(name="p2_sb", bufs=3) as s2:
                    dummy_ps = pp2.tile([1, 2], f32, tag="dummy", bufs=1)
                    for ht in range(HT):
                        pe_touch(h_all[ht][0:1, 0:2])
                    for dt in range(DT):
                        w2b = s2.tile([P, HT, P], f32r, tag="w2b")
                        nc.sync.dma_start(
                            out=w2b[:],
                            in_=w2T_d[:].rearrange("(k p) d -> p k d", p=P)[:, :, dt * P:(dt + 1) * P])
                        for (s0, sl) in slices:
                            py = pp2.tile([P, SLICE], f32, tag="py", space="PSUM")
                            for ht in range(HT):
                                nc.tensor.matmul(py[:, :sl], lhsT=w2b[:, ht, :],
                                                 rhs=h_all[ht][:, s0:s0 + sl],
                                                 start=(ht == 0), stop=(ht == HT - 1))
                            yT = s2.tile([P, SLICE], f32, tag="yT")
                            nc.vector.tensor_copy(out=yT[:, :sl], in_=py[:, :sl])
                            for i in range(sl // P):
                                g = (c0 + s0) // P + i
                                pyt = pp2.tile([P, P], f32, tag="pyt", space="PSUM", bufs=4)
                                nc.tensor.transpose(out=pyt[:], in_=yT[:, i * P:(i + 1) * P],
                                                    identity=ident[:])
                                ysub = s2.tile([P, P], f32, tag="ysub")
                                wcol = idw_sb[g - c0 // P][:, 1:2].bitcast(f32)
                                nc.vector.tensor_scalar_mul(ysub[:], pyt[:], wcol)
                                nc.sync.dma_start(
                                    out=y_d[P * g:P * (g + 1), dt * P:(dt + 1) * P],
                                    in_=ysub[:])

    nc.compile()
    return nc


def _marshal(x, gate_w, w1, w3, w2):
    xf = np.ascontiguousarray(x.reshape(T, D).astype(np.float32))
    xT = np.ascontiguousarray(xf.T)
    gwT = np.ascontiguousarray(gate_w.astype(np.float32).T)
    esel_all, w1T, w3T, w2T = [], [], [], []
    for e in range(E):
        sel = np.zeros((P, E), np.float32)
        sel[:, e] = 1.0
        esel_all.append(sel)
        w1T.append(np.ascontiguousarray(w1[e].astype(np.float32).T))
        w3T.append(np.ascontiguousarray(w3[e].astype(np.float32).T))
        w2T.append(np.ascontiguousarray(w2[e].astype(np.float32).T))
    consts = {
        "uexc": np.triu(np.ones((P, P), np.float32), 1),
        "ones_col": np.ones((P, 1), np.float32),
        "ones_row": np.ones((1, P), np.float32),
        "iota": (np.arange(P)[:, None] + P * np.arange(NB)[None, :]).astype(np.int32),
        "ident": np.eye(P, dtype=np.float32),
    }
    in_maps = []
    for e in range(E):
        in_maps.append({
            "x": xf, "xT": xT, "gwT": gwT,
            "w1T": w1T[e], "w3T": w3T[e], "w2T": w2T[e],
            "esel": esel_all[e], **consts,
        })
    return xf, in_maps


def _numpy_fallback(x, gate_w, w1, w3, w2):
    xf = x.reshape(T, D).astype(np.float64)
    logits = xf @ gate_w.astype(np.float64).T
    p = np.exp(logits - logits.max(1, keepdims=True))
    p /= p.sum(1, keepdims=True)
    idx = np.argsort(-p, axis=1, kind="stable")[:, :K]
    vals = np.take_along_axis(p, idx, 1)
    vals /= vals.sum(1, keepdims=True)
    y = np.zeros_like(xf)
    for e in range(E):
        m = (idx == e)
        wgt = (vals * m).sum(1)
        tsel = m.any(1)
        xe = xf[tsel]
        h = xe @ w1[e].astype(np.float64).T
        h = h / (1 + np.exp(-h)) * (xe @ w3[e].astype(np.float64).T)
        y[tsel] += wgt[tsel, None] * (h @ w2[e].astype(np.float64).T)
    return y.astype(np.float32).reshape(x.shape)


def run_spmd(x, gate_w, w1, w3, w2, trace=False):
    """Compile (cached), run on 8 cores, return (results, xf)."""
    from concourse.bass_utils import run_bass_kernel_spmd
    if "nc" not in _cache:
        _cache["nc"] = _build()
    _, in_maps = _marshal(x, gate_w, w1, w3, w2)
    res = run_bass_kernel_spmd(_cache["nc"], in_maps, list(range(E)), trace=trace)
    return res


def kernel(x, gate_w, w1, w3, w2):
    x = np.asarray(x)
    res = run_spmd(x, gate_w, w1, w3, w2)
    y = np.zeros((T, D), np.float32)
    for e in range(E):
        r = res.results[e]
        cnt = int(round(float(r["cnt"][0, 0])))
        if cnt > C:
            return _numpy_fallback(x, gate_w, w1, w3, w2)
        ids = r["idw"][:cnt, 0]
        rows = r["y_rows"][:cnt]
        if len(np.unique(ids)) == cnt:
            y[ids] += rows
        else:
            np.add.at(y, ids, rows)
    return y.reshape(x.shape)



# revision 3
# speedup vs baseline: 1.2649x; 1.2649x over previous
"""Trainium2 Bass kernel for an 8-expert top-2 SwiGLU MoE (expert parallelism).

Strategy (8 NeuronCores, one expert per core):
  - Every core receives the full token set, the gate, and ITS expert's weights.
  - On device, each core:
      1. computes gating logits for all 8192 tokens in f32r on the PE,
      2. finds the top-2 experts per token (vector.max), derives the softmax
         renormalized weight for its own expert, and a routed-token mask,
      3. turns the mask into compact slot indices via matmul prefix-sums,
      4. scatters (token_id, weight) pairs into a compact table with
         OOB-skipping indirect DMA, gathers the routed token rows (bf16),
      5. runs the SwiGLU FFN (x@w1T, x@w3T, silu*mul, @w2T) in bf16
         (fp32 PSUM accumulate) over the compacted tokens in ONE pass
         (weights streamed exactly once),
      6. writes feature-major output yT [D, C] (no on-device transpose
         or routing-weight scale).
  - The host scales each core's rows by the routing weight and adds them
    into the full output (expert-parallel combine).

Self-contained: hardcodes shapes for x[4,2048,1024], 8 experts, H=2816, top-2.
"""
import sys

sys.path.insert(0, "/opt/trn_rl_repo")

import numpy as np

# ---------------------------------------------------------------- config
B, S, D = 4, 2048, 1024
T = B * S                # 8192 tokens
E = 8                    # experts == cores
H = 2816
K = 2
P = 128
NB = T // P              # 64 token blocks (token = 128*b + p)
C = 2304                 # per-expert slot capacity (mean 2048, obs max 2175)
NG = C // P              # 18 slot tiles
HT = H // P              # 22
DT = D // P              # 8
GC = 512                 # gating chunk (tokens per gating matmul round)
NJ = T // GC             # 16
BPC = GC // P            # 4 token blocks per gating chunk
SLICES = [(0, 512), (512, 512), (1024, 512), (1536, 512), (2048, 256)]
GATE_F32R = True         # f32r (1-pass) gating matmuls; False -> exact fp32

_cache = {}


def _build():
    import concourse.bass as bass
    import concourse.bacc as bacc
    import concourse.mybir as mybir
    import concourse.tile as tile

    f32 = mybir.dt.float32
    f32r = mybir.dt.float32r
    bf16 = mybir.dt.bfloat16
    i32 = mybir.dt.int32
    Alu = mybir.AluOpType
    Act = mybir.ActivationFunctionType
    gdt = f32r if GATE_F32R else f32

    nc = bacc.Bacc("TRN2", target_bir_lowering=False, debug=False)

    # gating x, host-tiled so each chunk load is one contiguous 16KB
    # segment per partition: xTr[p, j*4096 + k*512 + c] = x[j*512+c, k*128+p]
    xTr_d = nc.dram_tensor("xTr", [P, NJ * DT * GC], gdt, kind="ExternalInput")
    xb_d = nc.dram_tensor("xb", [T, D], bf16, kind="ExternalInput")
    gwT_d = nc.dram_tensor("gwT", [D, E], gdt, kind="ExternalInput")
    # host-pre-tiled weights; per-tile loads are fully contiguous
    w1R_d = nc.dram_tensor("w1R", [HT * P, DT * P], bf16, kind="ExternalInput")
    w3R_d = nc.dram_tensor("w3R", [HT * P, DT * P], bf16, kind="ExternalInput")
    w2R_d = nc.dram_tensor("w2R", [DT * P, HT * P], bf16, kind="ExternalInput")
    esel_d = nc.dram_tensor("esel", [P, E], f32, kind="ExternalInput")
    uexc_d = nc.dram_tensor("uexc", [P, P], f32, kind="ExternalInput")
    onesc_d = nc.dram_tensor("ones_col", [P, 1], f32, kind="ExternalInput")
    onesr_d = nc.dram_tensor("ones_row", [1, P], f32, kind="ExternalInput")
    iota_d = nc.dram_tensor("iota", [P, NB], i32, kind="ExternalInput")
    ident_d = nc.dram_tensor("ident", [P, P], f32, kind="ExternalInput")
    identb_d = nc.dram_tensor("identb", [P, P], bf16, kind="ExternalInput")

    idw_d = nc.dram_tensor("idw", [C, 2], i32, kind="ExternalOutput")
    cnt_d = nc.dram_tensor("cnt", [1, 1], f32, kind="ExternalOutput")
    y_d = nc.dram_tensor("y_rows", [D, C], f32, kind="ExternalOutput")

    with tile.TileContext(nc) as tc:
        with tc.tile_pool(name="persist", bufs=1) as sp, \
             tc.tile_pool(name="wpool", bufs=1) as wp:
            # --- constants ---
            esel = sp.tile([P, E], f32)
            nc.sync.dma_start(out=esel[:], in_=esel_d[:])
            uexc = sp.tile([P, P], f32)
            nc.sync.dma_start(out=uexc[:], in_=uexc_d[:])
            onesc = sp.tile([P, 1], f32)
            nc.sync.dma_start(out=onesc[:], in_=onesc_d[:])
            onesr = sp.tile([1, P], f32)
            nc.sync.dma_start(out=onesr[:], in_=onesr_d[:])
            iota = sp.tile([P, NB], i32)
            nc.sync.dma_start(out=iota[:], in_=iota_d[:])
            ident = sp.tile([P, P], f32)
            nc.sync.dma_start(out=ident[:], in_=ident_d[:])
            identb = sp.tile([P, P], bf16)
            nc.sync.dma_start(out=identb[:], in_=identb_d[:])
            gw = sp.tile([P, DT, E], gdt)
            nc.sync.dma_start(out=gw[:], in_=gwT_d[:].rearrange("(k p) e -> p k e", p=P))

            # PE wait-absorber: matmul codegen allows a single sync wait, so
            # before any matmul that would need 2+ waits we make the PE observe
            # the extra semaphores through a tiny dummy matmul.
            dummy_ps = None

            def pe_touch(ap):
                # ap: [1, 1..2] SBUF region; result is garbage, absorbs one sem wait
                n = ap.shape[-1]
                nc.tensor.matmul(dummy_ps[0:1, 0:n], lhsT=ap[:, 0:1], rhs=ap,
                                 start=True, stop=True, skip_group_check=True)

            scores = sp.tile([P, NB * E], f32)     # [p, b*E+e] logits
            mx_all = sp.tile([P, NB * 8], f32)     # per-block top-8 (descending)
            se = sp.tile([P, NB], f32)
            incl_all = sp.tile([1, NB], f32)

            # ---------------- stage 1: gating logits ----------------
            with tc.tile_pool(name="gpsum", bufs=2, space="PSUM") as ppg, \
                 tc.tile_pool(name="gsb", bufs=3) as sg:
                dummy_ps = ppg.tile([1, 2], f32, tag="dummy", bufs=1)
                pe_touch(gw[0:1, 0, 0:2])
                pe_touch(ident[0:1, 0:2])
                pe_touch(identb[0:1, 0:2])
                pe_touch(uexc[0:1, 0:2])
                pe_touch(onesc[0:1, 0:1])
                pe_touch(onesr[0:1, 0:2])
                xTr3 = xTr_d[:].rearrange("p (j k c) -> p j k c", j=NJ, k=DT)
                for j in range(NJ):
                    xt = sg.tile([P, DT, GC], gdt, tag="xt", bufs=2)
                    nc.sync.dma_start(out=xt[:], in_=xTr3[:, j])
                    ps = ppg.tile([E, GC], f32, tag="ps", space="PSUM")
                    for k in range(DT):
                        nc.tensor.matmul(ps[:], lhsT=gw[:, k, :], rhs=xt[:, k, :],
                                         start=(k == 0), stop=(k == DT - 1))
                    sc_sb = sg.tile([E, GC], f32, tag="sc")
                    nc.vector.tensor_copy(out=sc_sb[:], in_=ps[:])
                    for i in range(BPC):
                        b = j * BPC + i
                        pst = ppg.tile([P, E], f32, tag="pst", space="PSUM")
                        nc.tensor.transpose(out=pst[:], in_=sc_sb[:, i * P:(i + 1) * P],
                                            identity=ident[0:E, 0:E])
                        nc.vector.tensor_copy(out=scores[:, b * E:(b + 1) * E], in_=pst[:])
                        blk = scores[:, b * E:(b + 1) * E]
                        nc.vector.max(out=mx_all[:, b * 8:(b + 1) * 8], in_=blk)
                        t8 = sg.tile([P, E], f32, tag="t8")
                        nc.vector.tensor_tensor(out=t8[:], in0=blk, in1=esel[:], op=Alu.mult)
                        nc.vector.reduce_sum(out=se[:, b:b + 1], in_=t8[:], axis=mybir.AxisListType.X)

                    # ---- routing for this chunk's BPC blocks (overlaps next chunk's PE) ----
                    b0 = j * BPC
                    mx3 = mx_all[:].rearrange("p (b e) -> p b e", e=8)
                    m1j = mx3[:, b0:b0 + BPC, 0]
                    m2j = mx3[:, b0:b0 + BPC, 1]
                    sej = se[:, b0:b0 + BPC]
                    dlt = sg.tile([P, BPC], f32, tag="dlt")
                    nc.vector.tensor_sub(out=dlt[:], in0=m2j, in1=m1j)
                    ed = sg.tile([P, BPC], f32, tag="ed")
                    nc.scalar.activation(out=ed[:], in_=dlt[:], func=Act.Exp)
                    den = sg.tile([P, BPC], f32, tag="den")
                    nc.vector.tensor_scalar_add(den[:], ed[:], 1.0)
                    wtop = sg.tile([P, BPC], f32, tag="wtop")
                    nc.vector.reciprocal(out=wtop[:], in_=den[:])
                    wsec = sg.tile([P, BPC], f32, tag="wsec")
                    nc.vector.tensor_scalar(out=wsec[:], in0=wtop[:], scalar1=-1.0, scalar2=1.0,
                                            op0=Alu.mult, op1=Alu.add)
                    istop = sg.tile([P, BPC], f32, tag="istop")
                    nc.vector.tensor_tensor(out=istop[:], in0=sej, in1=m1j, op=Alu.is_ge)
                    wdiff = sg.tile([P, BPC], f32, tag="wdiff")
                    nc.vector.tensor_sub(out=wdiff[:], in0=wtop[:], in1=wsec[:])
                    wE = sg.tile([P, BPC], f32, tag="wE")
                    nc.vector.tensor_tensor(out=wE[:], in0=istop[:], in1=wdiff[:], op=Alu.mult)
                    nc.vector.tensor_add(out=wE[:], in0=wE[:], in1=wsec[:])
                    maskj = sg.tile([P, BPC], f32, tag="maskj")
                    nc.vector.tensor_tensor(out=maskj[:], in0=sej, in1=m2j, op=Alu.is_ge)

                    pslot = ppg.tile([P, BPC], f32, tag="pslot", space="PSUM", bufs=1)
                    nc.tensor.matmul(pslot[:], lhsT=uexc[:], rhs=maskj[:], start=True, stop=False)
                    ptot = ppg.tile([1, BPC], f32, tag="dummy2", space="PSUM", bufs=1)
                    nc.tensor.matmul(ptot[:], lhsT=onesc[:], rhs=maskj[:], start=True, stop=True)
                    tot = sg.tile([1, BPC], f32, tag="tot")
                    nc.vector.tensor_copy(out=tot[:], in_=ptot[:])
                    init = 0.0 if j == 0 else incl_all[:, b0 - 1:b0]
                    nc.vector.tensor_tensor_scan(incl_all[:, b0:b0 + BPC], tot[:], tot[:], init,
                                                 op0=Alu.add, op1=Alu.bypass)
                    excl = sg.tile([1, BPC], f32, tag="excl")
                    nc.vector.tensor_sub(out=excl[:], in0=incl_all[:, b0:b0 + BPC], in1=tot[:])
                    nc.tensor.matmul(pslot[:], lhsT=onesr[:], rhs=excl[:], start=False, stop=True)
                    slot_f = sg.tile([P, BPC], f32, tag="slot_f")
                    nc.vector.tensor_copy(out=slot_f[:], in_=pslot[:])
                    off_f = sg.tile([P, BPC], f32, tag="off_f")
                    nc.vector.tensor_scalar(out=off_f[:], in0=maskj[:], scalar1=-1e6, scalar2=1e6,
                                            op0=Alu.mult, op1=Alu.add)
                    slot_oob = sg.tile([P, BPC], f32, tag="slot_oob")
                    nc.vector.tensor_add(out=slot_oob[:], in0=slot_f[:], in1=off_f[:])
                    slot_i = sg.tile([P, BPC], i32, tag="slot_i")
                    nc.vector.tensor_copy(out=slot_i[:], in_=slot_oob[:])
                    iw = sg.tile([P, 2 * BPC], i32, tag="iw")
                    iw3 = iw[:].rearrange("p (b two) -> p b two", two=2)
                    nc.vector.tensor_copy(out=iw3[:, :, 0], in_=iota[:, b0:b0 + BPC])
                    nc.vector.tensor_copy(out=iw3[:, :, 1], in_=wE[:].bitcast(i32))
                    for i in range(BPC):
                        nc.gpsimd.indirect_dma_start(
                            out=idw_d[:], out_offset=bass.IndirectOffsetOnAxis(ap=slot_i[:, i:i + 1], axis=0),
                            in_=iw[:, 2 * i:2 * i + 2], in_offset=None,
                            bounds_check=C - 1, oob_is_err=False)

                cnt_sb = sg.tile([1, 1], f32, tag="cnt")
                nc.vector.tensor_copy(out=cnt_sb[:], in_=incl_all[:, NB - 1:NB])
                nc.sync.dma_start(out=cnt_d[:], in_=cnt_sb[:])

            # ---------------- stage 2: gather + one-pass FFN ----------------
            with tc.tile_pool(name="ffn_sb", bufs=1) as sf:
                h_all = [sf.tile([P, C], bf16, tag=f"h{ht}", name=f"h{ht}") for ht in range(HT)]
                xgT = [sf.tile([P, C], bf16, tag=f"xgT{k}", name=f"xgT{k}") for k in range(DT)]
                idw_sb = [sf.tile([P, 2], i32, tag=f"idw{g}", name=f"idw{g}") for g in range(NG)]

                # gather routed token rows (bf16) and transpose to feature-major
                with tc.tile_pool(name="gat_ps", bufs=2, space="PSUM") as ppt, \
                     tc.tile_pool(name="gat_sb", bufs=3) as sgt:
                    dummy_ps = ppt.tile([1, 2], f32, tag="dummy", bufs=1)
                    for g in range(NG):
                        nc.sync.dma_start(out=idw_sb[g][:], in_=idw_d[P * g:P * (g + 1), :])
                        xg = sgt.tile([P, D], bf16, tag="xg", bufs=4)
                        nc.gpsimd.indirect_dma_start(
                            out=xg[:], out_offset=None, in_=xb_d[:],
                            in_offset=bass.IndirectOffsetOnAxis(ap=idw_sb[g][:, 0:1], axis=0))
                        for k in range(DT):
                            pst = ppt.tile([P, P], bf16, tag="pst", space="PSUM", bufs=4)
                            nc.tensor.transpose(out=pst[:], in_=xg[:, P * k:P * (k + 1)],
                                                identity=identb[:])
                            nc.vector.tensor_copy(out=xgT[k][:, g * P:(g + 1) * P], in_=pst[:])

                # FFN: pass1 h = silu(x@w1T) * (x@w3T); pass2 y = h @ w2T
                with tc.tile_pool(name="ffn_ps", bufs=2, space="PSUM") as pp1, \
                     tc.tile_pool(name="ffn_tmp", bufs=3) as s1:
                    dummy_ps = pp1.tile([1, 2], f32, tag="dummy", bufs=1)
                    for k in range(DT):
                        pe_touch(xgT[k][0:1, (NG - 1) * P:(NG - 1) * P + 2])
                    prev_silu = None
                    for ht in range(HT):
                        w1b = wp.tile([P, DT * P], bf16, tag="w1b", bufs=3)
                        nc.sync.dma_start(out=w1b[:], in_=w1R_d[ht * P:(ht + 1) * P, :])
                        w3b = wp.tile([P, DT * P], bf16, tag="w3b", bufs=3)
                        nc.sync.dma_start(out=w3b[:], in_=w3R_d[ht * P:(ht + 1) * P, :])
                        for (s0, sl) in SLICES:
                            ph1 = pp1.tile([P, 512], f32, tag="ph1", space="PSUM")
                            ph3 = pp1.tile([P, 512], f32, tag="ph3", space="PSUM")
                            for k in range(DT):
                                nc.tensor.matmul(ph1[:, :sl], lhsT=w1b[:, k * P:(k + 1) * P],
                                                 rhs=xgT[k][:, s0:s0 + sl],
                                                 start=(k == 0), stop=(k == DT - 1))
                            for k in range(DT):
                                nc.tensor.matmul(ph3[:, :sl], lhsT=w3b[:, k * P:(k + 1) * P],
                                                 rhs=xgT[k][:, s0:s0 + sl],
                                                 start=(k == 0), stop=(k == DT - 1))
                            silu = s1.tile([P, 512], f32, tag="silu")
                            nc.scalar.activation(out=silu[:, :sl], in_=ph1[:, :sl], func=Act.Silu)
                            nc.vector.tensor_tensor(out=h_all[ht][:, s0:s0 + sl],
                                                    in0=silu[:, :sl], in1=ph3[:, :sl], op=Alu.mult)
                            if prev_silu is not None:
                                pe_touch(prev_silu)
                            prev_silu = silu[0:1, 0:2]

                    for ht in range(HT):
                        pe_touch(h_all[ht][0:1, 0:2])
                    for dt in range(DT):
                        w2b = wp.tile([P, HT * P], bf16, tag="w2b", bufs=2)
                        nc.sync.dma_start(out=w2b[:], in_=w2R_d[dt * P:(dt + 1) * P, :])
                        for (s0, sl) in SLICES:
                            py = pp1.tile([P, 512], f32, tag="py", space="PSUM")
                            for ht in range(HT):
                                nc.tensor.matmul(py[:, :sl], lhsT=w2b[:, ht * P:(ht + 1) * P],
                                                 rhs=h_all[ht][:, s0:s0 + sl],
                                                 start=(ht == 0), stop=(ht == HT - 1))
                            yb = s1.tile([P, 512], f32, tag="yb")
                            nc.vector.tensor_copy(out=yb[:, :sl], in_=py[:, :sl])
                            nc.sync.dma_start(
                                out=y_d[dt * P:(dt + 1) * P, s0:s0 + sl],
                                in_=yb[:, :sl])

    nc.compile()
    return nc


def _marshal(x, gate_w, w1, w3, w2):
    import ml_dtypes
    bf16 = ml_dtypes.bfloat16
    xf = np.ascontiguousarray(x.reshape(T, D).astype(np.float32))
    # xTr[p, j, k, c] = x[j*GC+c, k*128+p]
    xTr = np.ascontiguousarray(
        xf.reshape(NJ, GC, DT, P).transpose(3, 0, 2, 1).reshape(P, NJ * DT * GC))
    xb = np.ascontiguousarray(xf.astype(bf16))
    gwT = np.ascontiguousarray(gate_w.astype(np.float32).T)
    consts = {
        "esel": None,  # filled per expert
        "uexc": np.triu(np.ones((P, P), np.float32), 1),
        "ones_col": np.ones((P, 1), np.float32),
        "ones_row": np.ones((1, P), np.float32),
        "iota": (np.arange(P)[:, None] + P * np.arange(NB)[None, :]).astype(np.int32),
        "ident": np.eye(P, dtype=np.float32),
        "identb": np.eye(P, dtype=bf16),
    }
    in_maps = []
    for e in range(E):
        sel = np.zeros((P, E), np.float32)
        sel[:, e] = 1.0
        w1e = w1[e].astype(np.float32)
        w3e = w3[e].astype(np.float32)
        w2e = w2[e].astype(np.float32)
        # w1R[ht*128+p, k*128+c] = w1[e][ht*128+c, k*128+p]
        w1R = np.ascontiguousarray(
            w1e.reshape(HT, P, DT, P).transpose(0, 3, 2, 1).reshape(HT * P, DT * P).astype(bf16))
        w3R = np.ascontiguousarray(
            w3e.reshape(HT, P, DT, P).transpose(0, 3, 2, 1).reshape(HT * P, DT * P).astype(bf16))
        # w2R[dt*128+p, ht*128+c] = w2[e][dt*128+c, ht*128+p]
        w2R = np.ascontiguousarray(
            w2e.reshape(DT, P, HT, P).transpose(0, 3, 2, 1).reshape(DT * P, HT * P).astype(bf16))
        m = dict(consts)
        m["esel"] = sel
        m.update({"xTr": xTr, "xb": xb, "gwT": gwT,
                  "w1R": w1R, "w3R": w3R, "w2R": w2R})
        in_maps.append(m)
    return xf, in_maps


def _numpy_fallback(x, gate_w, w1, w3, w2):
    xf = x.reshape(T, D).astype(np.float64)
    logits = xf @ gate_w.astype(np.float64).T
    p = np.exp(logits - logits.max(1, keepdims=True))
    p /= p.sum(1, keepdims=True)
    idx = np.argsort(-p, axis=1, kind="stable")[:, :K]
    vals = np.take_along_axis(p, idx, 1)
    vals /= vals.sum(1, keepdims=True)
    y = np.zeros_like(xf)
    for e in range(E):
        m = (idx == e)
        wgt = (vals * m).sum(1)
        tsel = m.any(1)
        xe = xf[tsel]
        h = xe @ w1[e].astype(np.float64).T
        h = h / (1 + np.exp(-h)) * (xe @ w3[e].astype(np.float64).T)
        y[tsel] += wgt[tsel, None] * (h @ w2[e].astype(np.float64).T)
    return y.astype(np.float32).reshape(x.shape)


def run_spmd(x, gate_w, w1, w3, w2, trace=False):
    """Compile (cached), run on 8 cores, return results."""
    from concourse.bass_utils import run_bass_kernel_spmd
    if "nc" not in _cache:
        _cache["nc"] = _build()
    _, in_maps = _marshal(x, gate_w, w1, w3, w2)
    res = run_bass_kernel_spmd(_cache["nc"], in_maps, list(range(E)), trace=trace)
    return res


def kernel(x, gate_w, w1, w3, w2):
    x = np.asarray(x)
    res = run_spmd(x, gate_w, w1, w3, w2)
    y = np.zeros((T, D), np.float32)
    for e in range(E):
        r = res.results[e]
        cnt = int(round(float(r["cnt"][0, 0])))
        if cnt > C:
            return _numpy_fallback(x, gate_w, w1, w3, w2)
        ids = r["idw"][:cnt, 0]
        w = r["idw"][:cnt, 1].view(np.float32)
        rows = w[:, None] * np.ascontiguousarray(r["y_rows"][:, :cnt].T)
        if len(np.unique(ids)) == cnt:
            y[ids] += rows
        else:
            np.add.at(y, ids, rows)
    return y.reshape(x.shape)
